# revision 2
# baseline (speedup 1.0000x reference)
"""Trainium2 Bass kernel for ContactMapPredictor (v3).

Computes, for B=2, N1=500, N2=800, D=128:
    p1 = h1 @ W1[:D] + b1 ; p2 = h2 @ W1[D:]
    hidden[b,n,m,:] = relu(p1[b,n,:] + p2[b,m,:])
    pred[b,n,m]     = hidden[b,n,m,:] @ W2 + b2
    mask[b,n,m]     = (S1[b,n]!=0) * (S2[b,m]!=0)
    y[b,n,m]        = (contact_map[b,n,m] < 0.5) * mask[b,n,m]
Returns (pred, y, mask) each reshaped [B, N1*N2].

v3 layout: the B*N1 = 1000 global rows are sharded 125/core across 8 cores;
125*8 = 1000 exactly, so there are no pad rows and each core owns rows of a
single batch (cores 0-3 -> b=0, 4-7 -> b=1), which also halves the h2
traffic (each core loads only its batch's h2). Per row, hid = relu(p2 +
p1col) is one fused DVE tensor_scalar (4x mode) or ACT activation over
[128, 800] bf16; the W2 reduce uses the shifted w2g trick so 32 rows
accumulate into one PSUM partition group (4 groups: 32/32/32/29 rows).

Differences from v2, each motivated by trace/slope measurements:
- ACT table preloaded at t~0 by a dummy activation (the 1283ns load was on
  the critical path to the first hid row).
- No PE warmup matmuls: the sim's p-state model keys on wall time (>3us =
  full speed), and the real loop matmuls start after that anyway; warmups
  only delayed the projection matmuls in PE program order.
- p2 psum->bf16 copy and the pred drains moved DVE->ACT, p1 bias-add moved
  ACT->DVE: on HW the loop is DVE-bound (tensor_scalar ~171ns vs sim 269),
  so DVE carries only the hid rows + tiny ops.
- contact_map is thresholded on the host ((cm < 0.5) -> 0/1 bf16 input,
  like the (S != 0) masks already were), killing the 400KB f32 cm DMA and
  the in-loop DVE compare; y_contact = t * mask is one 4x-mode DVE mult.
- mask / y_contact / pred all leave the device as bf16 (exact for 0/1
  values; pred rounding adds ~2e-3 rel err, well within budget). b2 is
  added on the host (it's a scalar), so drains are pure Identity converts
  and pred needs no bias tile.
- Four quarter drains (after rows 31/63/95/124) instead of two half drains:
  only the last 29-row group's drain+DMA sits in the tail.
- Non-critical DMAs (m1g/m2r/t in, mask/ycon out) ride the gpsimd SWDGE
  queue; the Pool engine is otherwise idle and its dma seq cost is tiny.

ACT_LANES: rows r with r%32 in ACT_LANES compute hid on ACT instead of DVE
(4 rows per lane; lanes >= 29 cover 3 rows). Tuned by hw slope benches.
"""

import numpy as np
import ml_dtypes

import bass_rust
import concourse.bass as bass
import concourse.tile as tile
import concourse.mybir as mybir
from concourse.bass_utils import run_bass_kernel_spmd

BF16NP = ml_dtypes.bfloat16
F32 = mybir.dt.float32
BF16 = mybir.dt.bfloat16

B, N1, N2, D = 2, 500, 800, 128
NCORES = 8
ROWS = 125            # global rows per core; 8*125 == B*N1 exactly
TH = 0.5
CHUNKS = ((0, 512), (512, 800))  # psum-bank-sized free-dim chunks
GROUP_STOPS = (31, 63, 95, 124)  # last row of each 32-row psum group

ACT_LANES = frozenset({4, 8, 13, 17, 21, 25, 29})


def _split_waits(nc):
    """This container's walrus build accepts at most ONE sync-wait command
    per instruction (any extra raises 'Too many sync wait commands' in
    codegen). Tile routinely attaches 2-3 waits to an instruction. Hoist
    all but the last wait onto same-engine NoOp carriers placed directly
    before the instruction — same-sequencer program order preserves the
    happens-before semantics exactly."""
    for blk in nc.m.functions[0].blocks:
        new = []
        for inst in blk.instructions:
            si = inst.sync_info
            waits = list(si.on_wait) if si and si.on_wait else []
            if len(waits) > 1 and inst.engine != mybir.EngineType.Unassigned:
                for w in waits[:-1]:
                    nop = mybir.InstNoOp(
                        name=nc.get_next_instruction_name(), engine=inst.engine
                    )
                    nop.sync_info = bass_rust.SyncInfo(on_wait=[w], on_update=[])
                    nc.register_instruction(nop)
                    new.append(nop)
                si.on_wait = waits[-1:]
                inst.sync_info = si
            new.append(inst)
        blk.instructions = new


def build_nc(repeat=1, act_lanes=None):
    """repeat>1 re-runs the main loop body (benchmarking aid: the slope of
    wall time vs repeat isolates device execution time from the per-call
    PJRT/axon dispatch overhead)."""
    if act_lanes is None:
        act_lanes = ACT_LANES
    nc = bass.Bass("TRN2", target_bir_lowering=False, debug=False)

    h2t_d = nc.declare_dram_parameter("h2t", [D, N2], BF16, isOutput=False)
    w1a_d = nc.declare_dram_parameter("w1a", [D, D], BF16, isOutput=False)
    w1b_d = nc.declare_dram_parameter("w1b", [D, D], BF16, isOutput=False)
    w2g_d = nc.declare_dram_parameter("w2g", [D, 63], BF16, isOutput=False)
    b1c_d = nc.declare_dram_parameter("b1c", [D, 1], F32, isOutput=False)
    h1t_d = nc.declare_dram_parameter("h1t", [D, ROWS], BF16, isOutput=False)
    m1g_d = nc.declare_dram_parameter("m1g", [1, ROWS], BF16, isOutput=False)
    m2r_d = nc.declare_dram_parameter("m2r", [1, N2], BF16, isOutput=False)
    tb_d = nc.declare_dram_parameter("tb", [ROWS, N2], BF16, isOutput=False)

    pred_d = nc.declare_dram_parameter("pred", [ROWS, N2], BF16, isOutput=True)
    mask_d = nc.declare_dram_parameter("mask", [ROWS, N2], BF16, isOutput=True)
    ycon_d = nc.declare_dram_parameter("ycon", [ROWS, N2], BF16, isOutput=True)

    with tile.TileContext(nc) as tc:
        with (
            tc.tile_pool(name="const", bufs=1) as const,
            tc.tile_pool(name="sb", bufs=1) as sb,
            tc.tile_pool(name="hidp", bufs=5) as hidp,
            tc.tile_pool(name="hidap", bufs=3) as hidap,
            tc.tile_pool(name="pps", bufs=1, space="PSUM") as pps,
        ):
            # ---- ACT table preload: a 1-element Identity at t~0 eats the
            # 1283ns table load off the critical path (Identity, Copy and
            # Relu share the exp_and_others table). ----
            zc = const.tile([128, 1], F32)
            nc.vector.memset(zc[:], 0.0)
            dumb = const.tile([1, 1], BF16)
            nc.scalar.activation(
                out=dumb[:], in_=zc[0:1, :],
                func=mybir.ActivationFunctionType.Identity,
            )

            # ---- critical-path DMAs on the SP HWDGE queue, in need order ----
            w1b = const.tile([D, D], BF16)
            nc.sync.dma_start(out=w1b[:], in_=w1b_d[:])
            # h2 in two tiles so the first p2 matmul waits only on chunk 0
            # (tile-level dependencies are whole-tile, not region)
            h2a = sb.tile([D, 512], BF16, tag="h2a")
            nc.sync.dma_start(out=h2a[:], in_=h2t_d[:, 0:512])
            h2b = sb.tile([D, 288], BF16, tag="h2b")
            nc.sync.dma_start(out=h2b[:], in_=h2t_d[:, 512:N2])
            w2g = const.tile([D, 63], BF16)
            nc.sync.dma_start(out=w2g[:], in_=w2g_d[:])
            # p1-path and mask-path inputs on the gpsimd SWDGE queue: keeps
            # the SP queue clear for the critical p2 path (Pool is idle)
            w1a = const.tile([D, D], BF16)
            nc.gpsimd.dma_start(out=w1a[:], in_=w1a_d[:])
            h1sb = sb.tile([D, ROWS], BF16, tag="h1sb")
            nc.gpsimd.dma_start(out=h1sb[:], in_=h1t_d[:])
            b1c = const.tile([D, 1], F32)
            nc.gpsimd.dma_start(out=b1c[:], in_=b1c_d[:])
            m1g = const.tile([1, ROWS], BF16)
            nc.gpsimd.dma_start(out=m1g[:], in_=m1g_d[:])
            m2r = const.tile([1, N2], BF16)
            nc.gpsimd.dma_start(out=m2r[:], in_=m2r_d[:])
            tb = sb.tile([ROWS, N2], BF16, tag="tb")
            nc.gpsimd.dma_start(out=tb[:], in_=tb_d[:])

            # ---- projections: p2T = W1b^T @ h2T ; p1bT = W1a^T @ h1T + b1 ----
            # Per-chunk psum tiles: the chunk-0 copy then waits only on the
            # chunk-0 matmul, not on both (tile-granular deps).
            p2ps_a = pps.tile([D, 512], F32, tag="p2a")
            nc.tensor.matmul(
                out=p2ps_a[:], lhsT=w1b[:], rhs=h2a[:], start=True, stop=True
            )
            p2ps_b = pps.tile([D, 288], F32, tag="p2b")
            nc.tensor.matmul(
                out=p2ps_b[:], lhsT=w1b[:], rhs=h2b[:], start=True, stop=True
            )
            p2sb = sb.tile([D, N2], BF16, tag="p2sb")
            nc.scalar.activation(
                out=p2sb[:, 0:512], in_=p2ps_a[:],
                func=mybir.ActivationFunctionType.Identity,
            )
            nc.scalar.activation(
                out=p2sb[:, 512:N2], in_=p2ps_b[:],
                func=mybir.ActivationFunctionType.Identity,
            )

            # p1ps reuses the p2a bank (free once the chunk-0 copy has read it)
            p1ps = pps.tile([D, 512], F32, tag="p2a", name="p1ps")
            nc.tensor.matmul(
                out=p1ps[:, 0:ROWS], lhsT=w1a[:], rhs=h1sb[:],
                start=True, stop=True,
            )
            p1b = sb.tile([D, ROWS], F32, tag="p1b")
            nc.vector.tensor_scalar(
                out=p1b[:], in0=p1ps[:, 0:ROWS], scalar1=b1c, scalar2=None,
                op0=mybir.AluOpType.add,
            )

            # mask psum gets its own banks; its rank-1 matmuls are emitted
            # inside the loop (after row 1) once m1g/m2r have landed
            mask_ps = pps.tile([D, N2], F32, tag="maskps", name="mask_ps")
            maskb = sb.tile([ROWS, N2], BF16, tag="maskb")

            # ---- main loop ----
            pred_ps = [
                pps.tile([64, N2], F32, tag="pred0", name="pred_ps0"),
                pps.tile([64, N2], F32, tag="pred1", name="pred_ps1"),
            ]
            predb = [
                sb.tile([64, N2], BF16, tag="predb0", name="predb0"),
                sb.tile([64, N2], BF16, tag="predb1", name="predb1"),
            ]
            # Drains and maskb are emitted on ACT *after* the next group's
            # first ACT hid row: ACT runs in program order, and a drain
            # sitting between two hid rows would delay the next hid past the
            # moment PE needs it, stalling the matmul stream ~0.85us per
            # group boundary.
            pending_act = []  # deferred (drain-or-maskb) closures
            mask_emitted = [False]

            def emit_maskb():
                nc.scalar.activation(
                    out=maskb[:], in_=mask_ps[0:ROWS, :],
                    func=mybir.ActivationFunctionType.Identity,
                )
                nc.gpsimd.dma_start(out=mask_d[:], in_=maskb[:])

            # group g -> psum tile g%2, region (g//2)*32: alternating tiles
            # means a group's first (start=True) matmul never carries a
            # tile-granular WAR wait on the previous group's drain read
            def emit_drain(g):
                half, goff = g % 2, (g // 2) * 32
                nrows = 29 if g == 3 else 32
                nc.scalar.activation(
                    out=predb[half][goff:goff + nrows, :],
                    in_=pred_ps[half][goff:goff + nrows, :],
                    func=mybir.ActivationFunctionType.Identity,
                )
                nc.sync.dma_start(
                    out=pred_d[g * 32:g * 32 + nrows, :],
                    in_=predb[half][goff:goff + nrows, :],
                )

            for rep in range(repeat):
              for r in range(ROWS):
                  g = r // 32
                  half = g % 2
                  goff = (g // 2) * 32
                  lane = r % 32
                  stop_lane = 28 if g == 3 else 31
                  col = p1b[:, r:r + 1]
                  if lane in act_lanes:
                      hid = hidap.tile([D, N2], BF16, tag="hida", name="hida")
                      nc.scalar.activation(
                          out=hid[:], in_=p2sb[:],
                          func=mybir.ActivationFunctionType.Relu,
                          bias=col, scale=1.0,
                      )
                      if rep == 0 and not mask_emitted[0]:
                          emit_maskb()
                          mask_emitted[0] = True
                      while pending_act:
                          pending_act.pop(0)()
                  else:
                      hid = hidp.tile([D, N2], BF16, tag="hid", name="hid")
                      nc.vector.tensor_scalar(
                          out=hid[:], in0=p2sb[:], scalar1=col, scalar2=0.0,
                          op0=mybir.AluOpType.add, op1=mybir.AluOpType.max,
                      )
                  lhsT = w2g[:, 31 - lane:63 - lane]
                  for lo, hi in CHUNKS:
                      nc.tensor.matmul(
                          out=pred_ps[half][goff:goff + 32, lo:hi],
                          lhsT=lhsT, rhs=hid[:, lo:hi],
                          start=(lane == 0), stop=(lane == stop_lane),
                          skip_group_check=True,
                      )
                  if rep == 0 and r == 1:
                      for lo, hi in CHUNKS:
                          nc.tensor.matmul(
                              out=mask_ps[0:ROWS, lo:hi], lhsT=m1g[:],
                              rhs=m2r[:, lo:hi],
                              start=True, stop=True, skip_group_check=True,
                          )
                  # y_contact = t * mask, one 4x-mode DVE mult; emitted a few
                  # rows in so the DVE never stalls waiting on maskb (ACT)
                  if rep == 0 and r == 10:
                      if not mask_emitted[0]:
                          emit_maskb()
                          mask_emitted[0] = True
                      yconb = sb.tile([ROWS, N2], BF16, tag="yconb")
                      nc.vector.tensor_tensor(
                          out=yconb[:], in0=tb[:], in1=maskb[:],
                          op=mybir.AluOpType.mult,
                      )
                      nc.gpsimd.dma_start(out=ycon_d[:], in_=yconb[:])
                  # drain each finished psum group while the next still fills;
                  # only the last group's drain is emitted inline (tail)
                  if rep == repeat - 1 and r in GROUP_STOPS:
                      if r == GROUP_STOPS[-1]:
                          emit_drain(g)
                      else:
                          pending_act.append(
                              (lambda gg: lambda: emit_drain(gg))(g))

    _split_waits(nc)
    return nc


def _marshal(inputs):
    """Full inputs -> list of 8 per-core input maps."""
    S1 = np.asarray(inputs["S_mol1"])                       # [B, N1]
    S2 = np.asarray(inputs["S_mol2"])                       # [B, N2]
    h1 = np.asarray(inputs["h_mol1"], dtype=np.float32)     # [B, N1, D]
    h2 = np.asarray(inputs["h_mol2"], dtype=np.float32)     # [B, N2, D]
    cm = np.asarray(inputs["contact_map"], dtype=np.float32)
    W1 = np.asarray(inputs["W1"], dtype=np.float32)         # [2D, D]
    b1 = np.asarray(inputs["b1"], dtype=np.float32)         # [D]
    W2 = np.asarray(inputs["W2"], dtype=np.float32)         # [D, 1]

    m1 = (S1 != 0).astype(np.float32).reshape(B * N1)
    h1f = h1.reshape(B * N1, D)
    tf = (cm < TH).astype(BF16NP).reshape(B * N1, N2)

    h2t = np.ascontiguousarray(h2.transpose(0, 2, 1)).astype(BF16NP)  # [B,D,N2]
    w1a = np.ascontiguousarray(W1[:D]).astype(BF16NP)
    w1b = np.ascontiguousarray(W1[D:]).astype(BF16NP)
    w2g = np.zeros((D, 63), np.float32)
    w2g[:, 31] = W2[:, 0]
    w2g = np.ascontiguousarray(w2g).astype(BF16NP)
    b1c = np.ascontiguousarray(b1.reshape(D, 1))
    m2r = (S2 != 0).astype(BF16NP)                          # [B, N2]

    in_maps = []
    for c in range(NCORES):
        bcore = c // 4
        sl = slice(c * ROWS, (c + 1) * ROWS)
        h1t = np.ascontiguousarray(h1f[sl].T).astype(BF16NP)      # [D, ROWS]
        m1g = np.ascontiguousarray(m1[sl].reshape(1, ROWS)).astype(BF16NP)
        in_maps.append({
            "h2t": np.ascontiguousarray(h2t[bcore]),
            "w1a": w1a, "w1b": w1b, "w2g": w2g, "b1c": b1c,
            "h1t": h1t, "m1g": m1g,
            "m2r": np.ascontiguousarray(m2r[bcore:bcore + 1]),
            "tb": np.ascontiguousarray(tf[sl]),
        })
    return in_maps


def _gather(results, b2=0.0):
    """Per-core outputs -> full-shape tuple (pred, y, mask)."""
    outs = []
    for name in ("pred", "ycon", "mask"):
        per_core = np.stack([results[c][name] for c in range(NCORES)])
        full = per_core.reshape(B, N1, N2).astype(np.float32)
        if name == "pred":
            full = full + b2
        outs.append(np.ascontiguousarray(full.reshape(B, N1 * N2)))
    pred, ycon, mask = outs
    return pred, ycon, mask


_NC_CACHE = None


def get_nc():
    global _NC_CACHE
    if _NC_CACHE is None:
        _NC_CACHE = build_nc()
    return _NC_CACHE


def kernel(**inputs):
    nc = get_nc()
    in_maps = _marshal(inputs)
    res = run_bass_kernel_spmd(nc, in_maps, core_ids=list(range(NCORES)))
    b2 = float(np.asarray(inputs["b2"], dtype=np.float32).reshape(-1)[0])
    return _gather(res.results, b2)


# revision 3
# speedup vs baseline: 1.2129x; 1.2129x over previous
"""Trainium2 Bass kernel for ContactMapPredictor (v3).

Computes, for B=2, N1=500, N2=800, D=128:
    p1 = h1 @ W1[:D] + b1 ; p2 = h2 @ W1[D:]
    hidden[b,n,m,:] = relu(p1[b,n,:] + p2[b,m,:])
    pred[b,n,m]     = hidden[b,n,m,:] @ W2 + b2
    mask[b,n,m]     = (S1[b,n]!=0) * (S2[b,m]!=0)
    y[b,n,m]        = (contact_map[b,n,m] < 0.5) * mask[b,n,m]
Returns (pred, y, mask) each reshaped [B, N1*N2].

v3 layout: the B*N1 = 1000 global rows are sharded 125/core across 8 cores;
125*8 = 1000 exactly, so there are no pad rows and each core owns rows of a
single batch (cores 0-3 -> b=0, 4-7 -> b=1), which also halves the h2
traffic (each core loads only its batch's h2). Per row, hid = relu(p2 +
p1col) is one fused DVE tensor_scalar (4x mode) or ACT activation over
[128, 800] bf16; the W2 reduce uses the shifted w2g trick so 32 rows
accumulate into one PSUM partition group (4 groups: 32/32/32/29 rows).

Differences from v2, each motivated by trace/slope measurements:
- ACT table preloaded at t~0 by a dummy activation (the 1283ns load was on
  the critical path to the first hid row).
- No PE warmup matmuls: the sim's p-state model keys on wall time (>3us =
  full speed), and the real loop matmuls start after that anyway; warmups
  only delayed the projection matmuls in PE program order.
- p2 psum->bf16 copy and the pred drains moved DVE->ACT, p1 bias-add moved
  ACT->DVE: on HW the loop is DVE-bound (tensor_scalar ~171ns vs sim 269),
  so DVE carries only the hid rows + tiny ops.
- contact_map is thresholded on the host ((cm < 0.5) -> 0/1 bf16 input,
  like the (S != 0) masks already were), killing the 400KB f32 cm DMA and
  the in-loop DVE compare; y_contact = t * mask is one 4x-mode DVE mult.
- mask / y_contact / pred all leave the device as bf16 (exact for 0/1
  values; pred rounding adds ~2e-3 rel err, well within budget). b2 is
  added on the host (it's a scalar), so drains are pure Identity converts
  and pred needs no bias tile.
- Four quarter drains (after rows 31/63/95/124) instead of two half drains:
  only the last 29-row group's drain+DMA sits in the tail.
- Non-critical DMAs (m1g/m2r/t in, mask/ycon out) ride the gpsimd SWDGE
  queue; the Pool engine is otherwise idle and its dma seq cost is tiny.

ACT_LANES: rows r with r%32 in ACT_LANES compute hid on ACT instead of DVE
(4 rows per lane; lanes >= 29 cover 3 rows). Tuned by hw slope benches.
"""

import numpy as np
import ml_dtypes

import bass_rust
import concourse.bass as bass
import concourse.tile as tile
import concourse.mybir as mybir
from concourse.bass_utils import run_bass_kernel_spmd

BF16NP = ml_dtypes.bfloat16
F32 = mybir.dt.float32
BF16 = mybir.dt.bfloat16

B, N1, N2, D = 2, 500, 800, 128
NCORES = 8
ROWS = 125            # global rows per core; 8*125 == B*N1 exactly
TH = 0.5
CHUNKS = ((0, 512), (512, 800))  # psum-bank-sized free-dim chunks
GROUP_STOPS = (31, 63, 95, 124)  # last row of each 32-row psum group

ACT_LANES = frozenset({4, 8, 13, 17, 21, 25, 29})


def _split_waits(nc):
    """This container's walrus build accepts at most ONE sync-wait command
    per instruction (any extra raises 'Too many sync wait commands' in
    codegen). Tile routinely attaches 2-3 waits to an instruction. Hoist
    all but the last wait onto same-engine NoOp carriers placed directly
    before the instruction — same-sequencer program order preserves the
    happens-before semantics exactly."""
    for blk in nc.m.functions[0].blocks:
        new = []
        for inst in blk.instructions:
            si = inst.sync_info
            waits = list(si.on_wait) if si and si.on_wait else []
            if len(waits) > 1 and inst.engine != mybir.EngineType.Unassigned:
                for w in waits[:-1]:
                    nop = mybir.InstNoOp(
                        name=nc.get_next_instruction_name(), engine=inst.engine
                    )
                    nop.sync_info = bass_rust.SyncInfo(on_wait=[w], on_update=[])
                    nc.register_instruction(nop)
                    new.append(nop)
                si.on_wait = waits[-1:]
                inst.sync_info = si
            new.append(inst)
        blk.instructions = new


def build_nc(repeat=1, act_lanes=None):
    """repeat>1 re-runs the main loop body (benchmarking aid: the slope of
    wall time vs repeat isolates device execution time from the per-call
    PJRT/axon dispatch overhead)."""
    if act_lanes is None:
        act_lanes = ACT_LANES
    nc = bass.Bass("TRN2", target_bir_lowering=False, debug=False)

    h2t_d = nc.declare_dram_parameter("h2t", [D, N2], BF16, isOutput=False)
    w1a_d = nc.declare_dram_parameter("w1a", [D, D], BF16, isOutput=False)
    w1b_d = nc.declare_dram_parameter("w1b", [D, D], BF16, isOutput=False)
    w2g_d = nc.declare_dram_parameter("w2g", [D, 63], BF16, isOutput=False)
    b1c_d = nc.declare_dram_parameter("b1c", [D, 1], F32, isOutput=False)
    h1t_d = nc.declare_dram_parameter("h1t", [D, ROWS], BF16, isOutput=False)
    m1g_d = nc.declare_dram_parameter("m1g", [1, ROWS], BF16, isOutput=False)
    m2r_d = nc.declare_dram_parameter("m2r", [1, N2], BF16, isOutput=False)
    tb_d = nc.declare_dram_parameter("tb", [ROWS, N2], BF16, isOutput=False)

    pred_d = nc.declare_dram_parameter("pred", [ROWS, N2], BF16, isOutput=True)
    mask_d = nc.declare_dram_parameter("mask", [ROWS, N2], BF16, isOutput=True)
    ycon_d = nc.declare_dram_parameter("ycon", [ROWS, N2], BF16, isOutput=True)

    with tile.TileContext(nc) as tc:
        with (
            tc.tile_pool(name="const", bufs=1) as const,
            tc.tile_pool(name="sb", bufs=1) as sb,
            tc.tile_pool(name="hidp", bufs=5) as hidp,
            tc.tile_pool(name="hidap", bufs=3) as hidap,
            tc.tile_pool(name="pps", bufs=1, space="PSUM") as pps,
        ):
            # ---- ACT table preload: a 1-element Identity at t~0 eats the
            # 1283ns table load off the critical path (Identity, Copy and
            # Relu share the exp_and_others table). ----
            zc = const.tile([128, 1], F32)
            nc.vector.memset(zc[:], 0.0)
            dumb = const.tile([1, 1], BF16)
            nc.scalar.activation(
                out=dumb[:], in_=zc[0:1, :],
                func=mybir.ActivationFunctionType.Identity,
            )

            # ---- critical-path DMAs on the SP HWDGE queue, in need order
            # (tried: w1b on the ACT HWDGE queue in parallel — its larger
            # fixed costs land w1b LATER than the serial SP queue does) ----
            w1b = const.tile([D, D], BF16)
            nc.sync.dma_start(out=w1b[:], in_=w1b_d[:])
            # h2 in two tiles so the first p2 matmul waits only on chunk 0
            # (tile-level dependencies are whole-tile, not region)
            h2a = sb.tile([D, 512], BF16, tag="h2a")
            nc.sync.dma_start(out=h2a[:], in_=h2t_d[:, 0:512])
            h2b = sb.tile([D, 288], BF16, tag="h2b")
            nc.sync.dma_start(out=h2b[:], in_=h2t_d[:, 512:N2])
            w2g = const.tile([D, 63], BF16)
            nc.sync.dma_start(out=w2g[:], in_=w2g_d[:])
            # p1-path and mask-path inputs on the gpsimd SWDGE queue: keeps
            # the SP queue clear for the critical p2 path (Pool is idle)
            w1a = const.tile([D, D], BF16)
            nc.gpsimd.dma_start(out=w1a[:], in_=w1a_d[:])
            h1sb = sb.tile([D, ROWS], BF16, tag="h1sb")
            nc.gpsimd.dma_start(out=h1sb[:], in_=h1t_d[:])
            b1c = const.tile([D, 1], F32)
            nc.gpsimd.dma_start(out=b1c[:], in_=b1c_d[:])
            m1g = const.tile([1, ROWS], BF16)
            nc.gpsimd.dma_start(out=m1g[:], in_=m1g_d[:])
            m2r = const.tile([1, N2], BF16)
            nc.gpsimd.dma_start(out=m2r[:], in_=m2r_d[:])
            tb = sb.tile([ROWS, N2], BF16, tag="tb")
            nc.gpsimd.dma_start(out=tb[:], in_=tb_d[:])

            # ---- projections: p2T = W1b^T @ h2T ; p1bT = W1a^T @ h1T + b1 ----
            # Per-chunk psum tiles: the chunk-0 copy then waits only on the
            # chunk-0 matmul, not on both (tile-granular deps).
            p2ps_a = pps.tile([D, 512], F32, tag="p2a")
            nc.tensor.matmul(
                out=p2ps_a[:], lhsT=w1b[:], rhs=h2a[:], start=True, stop=True
            )
            p2ps_b = pps.tile([D, 288], F32, tag="p2b")
            nc.tensor.matmul(
                out=p2ps_b[:], lhsT=w1b[:], rhs=h2b[:], start=True, stop=True
            )
            p2sb = sb.tile([D, N2], BF16, tag="p2sb")
            nc.scalar.activation(
                out=p2sb[:, 0:512], in_=p2ps_a[:],
                func=mybir.ActivationFunctionType.Identity,
            )
            nc.scalar.activation(
                out=p2sb[:, 512:N2], in_=p2ps_b[:],
                func=mybir.ActivationFunctionType.Identity,
            )

            # p1ps reuses the p2a bank (free once the chunk-0 copy has read it)
            p1ps = pps.tile([D, 512], F32, tag="p2a", name="p1ps")
            nc.tensor.matmul(
                out=p1ps[:, 0:ROWS], lhsT=w1a[:], rhs=h1sb[:],
                start=True, stop=True,
            )
            p1b = sb.tile([D, ROWS], F32, tag="p1b")
            nc.vector.tensor_scalar(
                out=p1b[:], in0=p1ps[:, 0:ROWS], scalar1=b1c, scalar2=None,
                op0=mybir.AluOpType.add,
            )

            # mask psum gets its own banks; its rank-1 matmuls are emitted
            # inside the loop (after row 1) once m1g/m2r have landed
            mask_ps = pps.tile([D, N2], F32, tag="maskps", name="mask_ps")
            maskb = sb.tile([ROWS, N2], BF16, tag="maskb")

            # ---- main loop ----
            pred_ps = [
                pps.tile([64, N2], F32, tag="pred0", name="pred_ps0"),
                pps.tile([64, N2], F32, tag="pred1", name="pred_ps1"),
            ]
            predb = [
                sb.tile([64, N2], BF16, tag="predb0", name="predb0"),
                sb.tile([64, N2], BF16, tag="predb1", name="predb1"),
            ]
            # Drains and maskb are emitted on ACT *after* the next group's
            # first ACT hid row: ACT runs in program order, and a drain
            # sitting between two hid rows would delay the next hid past the
            # moment PE needs it, stalling the matmul stream ~0.85us per
            # group boundary.
            pending_act = []  # deferred (drain-or-maskb) closures
            mask_emitted = [False]

            def emit_maskb():
                nc.scalar.activation(
                    out=maskb[:], in_=mask_ps[0:ROWS, :],
                    func=mybir.ActivationFunctionType.Identity,
                )
                nc.gpsimd.dma_start(out=mask_d[:], in_=maskb[:])

            # group g -> psum tile g%2, region (g//2)*32: alternating tiles
            # means a group's first (start=True) matmul never carries a
            # tile-granular WAR wait on the previous group's drain read
            def emit_drain(g):
                half, goff = g % 2, (g // 2) * 32
                nrows = 29 if g == 3 else 32
                nc.scalar.activation(
                    out=predb[half][goff:goff + nrows, :],
                    in_=pred_ps[half][goff:goff + nrows, :],
                    func=mybir.ActivationFunctionType.Identity,
                )
                nc.sync.dma_start(
                    out=pred_d[g * 32:g * 32 + nrows, :],
                    in_=predb[half][goff:goff + nrows, :],
                )

            for rep in range(repeat):
              for r in range(ROWS):
                  g = r // 32
                  half = g % 2
                  goff = (g // 2) * 32
                  lane = r % 32
                  stop_lane = 28 if g == 3 else 31
                  col = p1b[:, r:r + 1]
                  if lane in act_lanes:
                      hid = hidap.tile([D, N2], BF16, tag="hida", name="hida")
                      nc.scalar.activation(
                          out=hid[:], in_=p2sb[:],
                          func=mybir.ActivationFunctionType.Relu,
                          bias=col, scale=1.0,
                      )
                      if rep == 0 and not mask_emitted[0]:
                          emit_maskb()
                          mask_emitted[0] = True
                      while pending_act:
                          pending_act.pop(0)()
                  else:
                      hid = hidp.tile([D, N2], BF16, tag="hid", name="hid")
                      nc.vector.tensor_scalar(
                          out=hid[:], in0=p2sb[:], scalar1=col, scalar2=0.0,
                          op0=mybir.AluOpType.add, op1=mybir.AluOpType.max,
                      )
                  lhsT = w2g[:, 31 - lane:63 - lane]
                  for lo, hi in CHUNKS:
                      nc.tensor.matmul(
                          out=pred_ps[half][goff:goff + 32, lo:hi],
                          lhsT=lhsT, rhs=hid[:, lo:hi],
                          start=(lane == 0), stop=(lane == stop_lane),
                          skip_group_check=True,
                      )
                  if rep == 0 and r == 1:
                      for lo, hi in CHUNKS:
                          nc.tensor.matmul(
                              out=mask_ps[0:ROWS, lo:hi], lhsT=m1g[:],
                              rhs=m2r[:, lo:hi],
                              start=True, stop=True, skip_group_check=True,
                          )
                  # y_contact = t * mask, one 4x-mode DVE mult; emitted a few
                  # rows in so the DVE never stalls waiting on maskb (ACT)
                  if rep == 0 and r == 10:
                      if not mask_emitted[0]:
                          emit_maskb()
                          mask_emitted[0] = True
                      yconb = sb.tile([ROWS, N2], BF16, tag="yconb")
                      nc.vector.tensor_tensor(
                          out=yconb[:], in0=tb[:], in1=maskb[:],
                          op=mybir.AluOpType.mult,
                      )
                      nc.gpsimd.dma_start(out=ycon_d[:], in_=yconb[:])
                  # drain each finished psum group while the next still fills;
                  # only the last group's drain is emitted inline (tail)
                  if rep == repeat - 1 and r in GROUP_STOPS:
                      if r == GROUP_STOPS[-1]:
                          emit_drain(g)
                      else:
                          pending_act.append(
                              (lambda gg: lambda: emit_drain(gg))(g))

    _split_waits(nc)
    return nc


def _marshal(inputs):
    """Full inputs -> list of 8 per-core input maps."""
    S1 = np.asarray(inputs["S_mol1"])                       # [B, N1]
    S2 = np.asarray(inputs["S_mol2"])                       # [B, N2]
    h1 = np.asarray(inputs["h_mol1"], dtype=np.float32)     # [B, N1, D]
    h2 = np.asarray(inputs["h_mol2"], dtype=np.float32)     # [B, N2, D]
    cm = np.asarray(inputs["contact_map"], dtype=np.float32)
    W1 = np.asarray(inputs["W1"], dtype=np.float32)         # [2D, D]
    b1 = np.asarray(inputs["b1"], dtype=np.float32)         # [D]
    W2 = np.asarray(inputs["W2"], dtype=np.float32)         # [D, 1]

    m1 = (S1 != 0).astype(np.float32).reshape(B * N1)
    h1f = h1.reshape(B * N1, D)
    tf = (cm < TH).astype(BF16NP).reshape(B * N1, N2)

    h2t = np.ascontiguousarray(h2.transpose(0, 2, 1)).astype(BF16NP)  # [B,D,N2]
    w1a = np.ascontiguousarray(W1[:D]).astype(BF16NP)
    w1b = np.ascontiguousarray(W1[D:]).astype(BF16NP)
    w2g = np.zeros((D, 63), np.float32)
    w2g[:, 31] = W2[:, 0]
    w2g = np.ascontiguousarray(w2g).astype(BF16NP)
    b1c = np.ascontiguousarray(b1.reshape(D, 1))
    m2r = (S2 != 0).astype(BF16NP)                          # [B, N2]

    in_maps = []
    for c in range(NCORES):
        bcore = c // 4
        sl = slice(c * ROWS, (c + 1) * ROWS)
        h1t = np.ascontiguousarray(h1f[sl].T).astype(BF16NP)      # [D, ROWS]
        m1g = np.ascontiguousarray(m1[sl].reshape(1, ROWS)).astype(BF16NP)
        in_maps.append({
            "h2t": np.ascontiguousarray(h2t[bcore]),
            "w1a": w1a, "w1b": w1b, "w2g": w2g, "b1c": b1c,
            "h1t": h1t, "m1g": m1g,
            "m2r": np.ascontiguousarray(m2r[bcore:bcore + 1]),
            "tb": np.ascontiguousarray(tf[sl]),
        })
    return in_maps


def _gather(results, b2=0.0):
    """Per-core outputs -> full-shape tuple (pred, y, mask)."""
    outs = []
    for name in ("pred", "ycon", "mask"):
        per_core = np.stack([results[c][name] for c in range(NCORES)])
        full = per_core.reshape(B, N1, N2).astype(np.float32)
        if name == "pred":
            full = full + b2
        outs.append(np.ascontiguousarray(full.reshape(B, N1 * N2)))
    pred, ycon, mask = outs
    return pred, ycon, mask


_NC_CACHE = None


def get_nc():
    global _NC_CACHE
    if _NC_CACHE is None:
        _NC_CACHE = build_nc()
    return _NC_CACHE


def kernel(**inputs):
    nc = get_nc()
    in_maps = _marshal(inputs)
    res = run_bass_kernel_spmd(nc, in_maps, core_ids=list(range(NCORES)))
    b2 = float(np.asarray(inputs["b2"], dtype=np.float32).reshape(-1)[0])
    return _gather(res.results, b2)
